# revision 26
# baseline (speedup 1.0000x reference)
"""Gated multi-head self-attention on 8 Trainium2 NeuronCores.

Sharding: 16 heads / 8 cores = 2 heads per core. Each core computes its two
heads end-to-end and writes a partial [NT, D] output (fp16); the host sums
the 8 partials and adds the head-summed output bias.

Device algorithm per core (heads h0, h1), fp16 matmul front-end, f32 PSUM:
  QT/KT[128, 4096]   = W_{q,k}.T @ x.T + b     (f16; heads stacked on partitions)
  V'[s, h, 66]       = [x@W_v | 1]             (f32r, via VT + PE transpose)
  S^T[s, q]          = KT.T @ QT               (f16 in, heads packed in PE quads)
  P[s, q]            = exp(0.125 * S^T)        (f32r out: Act f16 writes run at
                                                half rate, f32r at full rate)
  po[65, q]          = V'.T @ P                (row 64 = softmax denominators)
  osm[e, q]          = po[0:64]·bcast(1/po[64]) + bv    (f16)
  nsq[q]             = ones.T @ osm^2          (per-token squared norms)
  denom_h            = max(mean sqrt(nsq), 1e-5);  wo_sc = wo * gate/16 / denom
  out               += osm.T @ wo_sc  summed over 2 heads, stored fp16

Scheduling: batch-1 QKV projection is interleaved into batch-0 attention
j-loops as filler thunks so PE stays dense while Act streams the exp()
softmax; phase C overlaps the next chunk's j-loop via the engine queues.
DMA queues: x loads, drow row-moves and output stores all on SP, which is
otherwise idle in those phases.
"""

import sys

sys.path.insert(0, "/opt/trn_rl_repo")

import contextlib

import numpy as np

import concourse.bacc as bacc
import concourse.mybir as mybir
import concourse.tile as tile
from concourse.bass_utils import run_bass_kernel_spmd
from concourse.masks import make_identity

f32 = mybir.dt.float32
f32r = mybir.dt.float32r
f16 = mybir.dt.float16
AF = mybir.ActivationFunctionType
ALU = mybir.AluOpType

B, T, D, H, HD = 2, 2048, 1024, 16, 64
NCORES = 8
HPC = H // NCORES  # heads per core = 2
NT = B * T         # 4096 tokens
NJ = T // 128      # 16 key tiles per batch
SCALE = 1.0 / np.sqrt(HD)  # 0.125

_BUILD_CACHE = {}


def _build(with_mask: bool, repeat: int = 1):
    nc = bacc.Bacc(None, target_bir_lowering=False)

    xT = nc.declare_dram_parameter("xT", [D, NT], f16, isOutput=False)
    wqkv = nc.declare_dram_parameter("wqkv", [3, 8, 128, 128], f16, isOutput=False)
    bqk = nc.declare_dram_parameter("bqk", [2, 128], f32, isOutput=False)
    bv = nc.declare_dram_parameter("bv", [HPC, HD], f32, isOutput=False)
    wo = nc.declare_dram_parameter("wo", [HPC, HD, D], f16, isOutput=False)
    selc = nc.declare_dram_parameter("selc", [128, 128], f16, isOutput=False)
    sel64d = nc.declare_dram_parameter("sel64d", [128, 2], f32r, isOutput=False)
    outp = nc.declare_dram_parameter("outp", [NT, D], f16, isOutput=True)
    if with_mask:
        maskT = nc.declare_dram_parameter("maskT", [T, T], f32, isOutput=False)

    with tile.TileContext(nc) as tc, contextlib.ExitStack() as ctx:
        wp = ctx.enter_context(tc.tile_pool(name="wp", bufs=1))
        big = ctx.enter_context(tc.tile_pool(name="big", bufs=2))
        osmp = ctx.enter_context(tc.tile_pool(name="osmp", bufs=2))
        nsqp = ctx.enter_context(tc.tile_pool(name="nsqp", bufs=2))
        xp = ctx.enter_context(tc.tile_pool(name="xp", bufs=4))
        vtp = ctx.enter_context(tc.tile_pool(name="vtp", bufs=2))
        pp = ctx.enter_context(tc.tile_pool(name="pp", bufs=4))
        o65p = ctx.enter_context(tc.tile_pool(name="o65p", bufs=4))
        auxp = ctx.enter_context(tc.tile_pool(name="auxp", bufs=4))
        rowp = ctx.enter_context(tc.tile_pool(name="rowp", bufs=4))
        scp = ctx.enter_context(tc.tile_pool(name="scp", bufs=4))
        op = ctx.enter_context(tc.tile_pool(name="op", bufs=5))
        if with_mask:
            mp = ctx.enter_context(tc.tile_pool(name="mp", bufs=2))
            tmpp = ctx.enter_context(tc.tile_pool(name="tmpp", bufs=2))
        s2p = ctx.enter_context(tc.tile_pool(name="s2p", bufs=2, space="PSUM"))
        pot = ctx.enter_context(tc.tile_pool(name="pot", bufs=2, space="PSUM"))
        miscp = ctx.enter_context(tc.tile_pool(name="miscp", bufs=2, space="PSUM"))

        # ---- constants / weights ----
        wqkv_sb = wp.tile([128, 3, 8, 128], f16)
        nc.sync.dma_start(out=wqkv_sb[:], in_=wqkv.ap().rearrange("q d p m -> p q d m"))
        bqk_sb = wp.tile([128, 2], f32)
        nc.sync.dma_start(out=bqk_sb[:], in_=bqk.ap().rearrange("q p -> p q"))
        bv_sb = wp.tile([HD, HPC], f32)
        nc.sync.dma_start(out=bv_sb[:], in_=bv.ap().rearrange("h p -> p h"))
        wo_sb = wp.tile([128, D], f16)
        nc.sync.dma_start(out=wo_sb[:], in_=wo.ap().rearrange("h p d -> (h p) d"))
        wo_sc = wp.tile([128, D], f16)
        ones_f = wp.tile([128, 1], f32)
        nc.vector.memset(ones_f[:], 1.0)
        sel64 = wp.tile([128, 2], f32r)
        nc.sync.dma_start(out=sel64[:], in_=sel64d.ap())
        selbc = wp.tile([128, 128], f16)
        nc.sync.dma_start(out=selbc[:], in_=selc.ap())
        identb = wp.tile([128, 128], f16)
        make_identity(nc, identb[:])
        inv2w = wp.tile([128, 512], f16)
        nc.vector.memset(inv2w[:], 0.0)

        # V' [s-part, s-tile, head, 66]: cols 0:64 = V, col 64 = ones, 65 pad
        Vp = wp.tile([128, NT // 128, HPC, 66], f32r)
        nc.vector.tensor_copy(Vp[:, :, :, 64:65],
                              ones_f.broadcast_to([128, NT // 128, HPC, 1]))

        QT = big.tile([128, NT], f16, tag="big")
        KT = big.tile([128, NT], f16, tag="big")

        osm2 = osmp.tile([128, NT], f16, name="osm2", tag="osm")
        nsq2 = nsqp.tile([2, NT], f16, name="nsq2", tag="nsq")

        def qkv_thunks(c4, fine=False):
            """Emit-thunks projecting tokens [c4*1024, (c4+1)*1024)."""
            thunks = []
            xsh = {}

            def dma_thunk():
                if fine:
                    # per-sub tiles: sub 0's 8 d-slices land first, so its
                    # projection (and the first attention chunk) starts at
                    # half the chunk-load latency
                    for sub in range(2):
                        xs = xp.tile([128, 8, 512], f16, tag="xs",
                                     name=f"xsf{sub}")
                        for dd in range(8):
                            nc.sync.dma_start(
                                out=xs[:, dd, :],
                                in_=xT.ap()[:, c4 * 1024 + sub * 512:
                                            c4 * 1024 + (sub + 1) * 512]
                                .rearrange("(dc p) t -> p dc t", p=128)[:, dd, :])
                        xsh[f"f{sub}"] = xs
                else:
                    for hh in range(2):
                        xs = xp.tile([128, 4, 1024], f16, tag="xs",
                                     name=f"xs{hh}")
                        for dd in range(4):
                            nc.sync.dma_start(
                                out=xs[:, dd, :],
                                in_=xT.ap()[:, c4 * 1024:(c4 + 1) * 1024]
                                .rearrange("(dc p) t -> p dc t", p=128)[:, hh * 4 + dd, :])
                        xsh[hh] = xs
            thunks.append(dma_thunk)

            def xsl(dc, cols):
                if fine:
                    sub = 0 if cols.start < 512 else 1
                    return xsh[f"f{sub}"][:, dc, slice(cols.start - sub * 512,
                                                       cols.stop - sub * 512)]
                return xsh[dc // 4][:, dc % 4, cols]

            for sub in range(2):
                scols = slice(sub * 512, (sub + 1) * 512)
                gcols = slice(c4 * 1024 + sub * 512, c4 * 1024 + (sub + 1) * 512)

                def qk_proj(p, scols=scols, gcols=gcols):
                    dst = QT if p == 0 else KT
                    ps = miscp.tile([128, 512], f32, tag="a", name="ps_qk")
                    for dc in range(8):
                        nc.tensor.matmul(ps[:], wqkv_sb[:, p, dc, :], xsl(dc, scols),
                                         start=(dc == 0), stop=(dc == 7))
                    nc.vector.tensor_scalar_add(dst[:, gcols], ps[:],
                                                bqk_sb[:, p:p + 1])
                thunks.append(lambda p=0, f=qk_proj: f(p))
                thunks.append(lambda p=1, f=qk_proj: f(p))

                def v_proj(c4=c4, sub=sub, scols=scols):
                    psv = miscp.tile([128, 512], f32, tag="a", name="ps_v")
                    for dc in range(8):
                        nc.tensor.matmul(psv[:], wqkv_sb[:, 2, dc, :], xsl(dc, scols),
                                         start=(dc == 0), stop=(dc == 7))
                    vt = vtp.tile([128, 512], f16, tag="vt", name="vt")
                    nc.vector.tensor_copy(vt[:], psv[:])
                    xsh[f"vt{sub}"] = vt
                thunks.append(v_proj)

                def v_tr(s4, c4=c4, sub=sub):
                    vt = xsh[f"vt{sub}"]
                    j = c4 * 8 + sub * 4 + s4
                    for h in range(HPC):
                        pt = miscp.tile([128, 1024], f16, tag="a", name="ps_tr")
                        nc.tensor.transpose(
                            pt[:, 0:64],
                            vt[h * 64:(h + 1) * 64, s4 * 128:(s4 + 1) * 128],
                            identb[h * 64:(h + 1) * 64, h * 64:(h + 1) * 64])
                        nc.vector.tensor_copy(Vp[:, j, h, 0:64], pt[:, 0:64])
                for s4 in range(4):
                    thunks.append(lambda s4=s4, f=v_tr: f(s4))
            return thunks  # 15 thunks

        def attn_qc(b, qc, fillers=()):
            """One 512-query attention chunk, j-major AV, inline phase C."""
            qcols = slice(b * T + qc * 512, b * T + (qc + 1) * 512)
            po = [pot.tile([65, 512], f32, tag="po", name=f"po{h}")
                  for h in range(HPC)]
            prev_pe = None
            prev_j = -1
            nf = len(fillers)
            fi = 0

            def av(j, pe):
                for h in range(HPC):
                    nc.tensor.matmul(po[h][:], Vp[:, b * NJ + j, h, 0:65],
                                     pe[:, h * 512:(h + 1) * 512],
                                     start=(j == 0), stop=(j == NJ - 1))

            for j in range(NJ):
                scols = slice(b * T + j * 128, b * T + (j + 1) * 128)
                s2 = s2p.tile([128, 1024], f32, tag="s2", name="s2")
                for h in range(HPC):
                    nc.tensor.matmul(s2[:, h * 512:(h + 1) * 512],
                                     KT[h * 64:(h + 1) * 64, scols],
                                     QT[h * 64:(h + 1) * 64, qcols],
                                     start=True, stop=True,
                                     tile_position=(h * 64, 0))
                pe = pp.tile([128, 1024], f32r, tag="p", name="pe")
                if with_mask:
                    mt = mp.tile([128, 512], f32, tag="m", name="mt")
                    nc.sync.dma_start(
                        out=mt[:],
                        in_=maskT.ap()[j * 128:(j + 1) * 128,
                                       qc * 512:(qc + 1) * 512])
                    tmp = tmpp.tile([128, 1024], f32, tag="tmp", name="tmp")
                    for h in range(HPC):
                        nc.vector.scalar_tensor_tensor(
                            tmp[:, h * 512:(h + 1) * 512],
                            s2[:, h * 512:(h + 1) * 512], SCALE, mt[:],
                            op0=ALU.mult, op1=ALU.add)
                    nc.scalar.activation(pe[:], tmp[:], AF.Exp)
                else:
                    nc.scalar.activation(pe[:], s2[:], AF.Exp, scale=float(SCALE))
                if prev_pe is not None:
                    av(prev_j, prev_pe)
                prev_pe, prev_j = pe, j
                want = (j + 1) * nf // NJ
                while fi < want:
                    fillers[fi]()
                    fi += 1
            av(prev_j, prev_pe)

            # phase C inline: po's only reader is the o65 copy, so po frees
            # early; the rest overlaps the next chunk's j-loop via the queues
            for h in range(HPC):
                o65 = o65p.tile([65, 512], f32, tag="o65", name="o65")
                nc.vector.tensor_copy(o65[:], po[h][:])
                drow = rowp.tile([1, 512], f32, tag="row", name="drow")
                nc.sync.dma_start(out=drow[:], in_=o65[64:65, :])
                rrow = rowp.tile([1, 512], f32, tag="row", name="rrow")
                nc.vector.reciprocal(rrow[:], drow[:])
                bc = auxp.tile([HD, 512], f32, tag="bc", name="bc")
                nc.gpsimd.partition_broadcast(bc[:], rrow[:])
                t1 = auxp.tile([HD, 512], f32r, tag="t1", name="t1")
                nc.vector.tensor_tensor(t1[:], o65[0:64, :], bc[:], op=ALU.mult)
                oc = osm2[h * 64:(h + 1) * 64, qcols]
                nc.vector.tensor_scalar_add(oc, t1[:], bv_sb[:, h:h + 1])
            sq2 = auxp.tile([128, 512], f32r, tag="sq", name="sq2")
            nc.vector.tensor_tensor(sq2[:], osm2[:, qcols], osm2[:, qcols],
                                    op=ALU.mult)
            pn = miscp.tile([128, 512], f32, tag="a", name="ps_n")
            nc.tensor.matmul(pn[0:2, :], sel64[:], sq2[:], start=True, stop=True)
            nc.vector.tensor_copy(nsq2[:, qcols], pn[0:2, :])

        def tail():
            s1 = scp.tile([2, NT], f16, tag="s1", bufs=1, name="s1")
            tot2 = scp.tile([2, 1], f32, tag="c1", name="tot2")
            nc.scalar.activation(s1[:], nsq2[:], AF.Sqrt, accum_out=tot2[:])
            den2 = scp.tile([2, 1], f32, tag="c1", name="den2")
            nc.vector.tensor_scalar(den2[:], tot2[:], 1.0 / NT, 1e-5,
                                    op0=ALU.mult, op1=ALU.max)
            inv2 = scp.tile([2, 1], f32, tag="c1", name="inv2")
            nc.vector.reciprocal(inv2[:], den2[:])
            nc.vector.tensor_scalar(inv2w[0:2, :],
                                    ones_f[0:2, :].broadcast_to([2, 512]),
                                    inv2[:], None, op0=ALU.mult)
            pinv = miscp.tile([128, 512], f32, tag="a", name="ps_i")
            nc.tensor.matmul(pinv[:], selbc[:], inv2w[:], start=True, stop=True)
            inv128 = scp.tile([128, 1], f32, tag="c128", name="inv128")
            nc.vector.tensor_copy(inv128[:], pinv[:, 0:1])
            nc.vector.tensor_scalar(wo_sc[:], wo_sb[:], inv128[:],
                                    None, op0=ALU.mult)
            for t in range(NT // 128):
                trows = slice(t * 128, (t + 1) * 128)
                osb = op.tile([128, D], f16, tag="ob", name="osb")
                if t % 2 == 0:
                    big_ppj = s2p.tile([128, 1024], f32, tag="s2", name="ps_p")
                    halves = [big_ppj[:, 0:512], big_ppj[:, 512:1024]]
                else:
                    halves = [miscp.tile([128, 512], f32, tag="a",
                                         name=f"ps_p{d}")[:]
                              for d in range(2)]
                for dchunk in range(2):
                    dcols = slice(dchunk * 512, (dchunk + 1) * 512)
                    ppj = halves[dchunk]
                    nc.tensor.matmul(ppj, osm2[:, trows],
                                     wo_sc[:, dcols], start=True, stop=True)
                    # per-half copy+store: twice the chunks in flight; DVE
                    # gets the larger share (f16 copies: DVE 1.4us vs Act 2us)
                    if (2 * t + dchunk) % 5 < 3:
                        nc.vector.tensor_copy(osb[:, dcols], ppj)
                    else:
                        nc.scalar.activation(osb[:, dcols], ppj, AF.Copy)
                nc.sync.dma_start(out=outp.ap()[trows, :], in_=osb[:])

        def _emit_all():
            ch = [qkv_thunks(c, fine=(c == 0)) for c in range(4)]
            for t in ch[0]:
                t()
            ch[1][0]()  # prefetch chunk-1 x before attention starts
            attn_qc(0, 0, fillers=ch[1][1:] + [ch[2][0]])
            attn_qc(0, 1, fillers=ch[2][1:] + [ch[3][0]])
            attn_qc(0, 2, fillers=ch[3][1:])
            attn_qc(0, 3)
            for qc in range(4):
                attn_qc(1, qc)
            tail()

        if repeat > 1:
            with tc.For_i(0, repeat, 1):
                _emit_all()
        else:
            _emit_all()

    nc.compile()
    return nc


def _get_nc(with_mask: bool):
    key = with_mask
    if key not in _BUILD_CACHE:
        _BUILD_CACHE[key] = _build(with_mask)
    return _BUILD_CACHE[key]


def _make_in_maps(hidden_states, attn_mask, W_q, b_q, W_k, b_k, W_v, b_v,
                  W_o, b_o, gate, with_mask):
    x = hidden_states.reshape(NT, D)
    xT = np.ascontiguousarray(x.T.astype(np.float16))
    g = np.clip(gate, 0.0, 1.0)

    in_maps = []
    for c in range(NCORES):
        hs = slice(c * HPC, (c + 1) * HPC)
        wq = np.concatenate([W_q[c * HPC + i] for i in range(HPC)], axis=1)
        wk = np.concatenate([W_k[c * HPC + i] for i in range(HPC)], axis=1)
        wv = np.concatenate([W_v[c * HPC + i] for i in range(HPC)], axis=1)
        wqkv_c = np.ascontiguousarray(
            np.stack([wq, wk, wv], axis=0).reshape(3, 8, 128, 128)
            .astype(np.float16))
        bqk_c = np.ascontiguousarray(np.stack(
            [np.concatenate([b_q[c * HPC + i] for i in range(HPC)]),
             np.concatenate([b_k[c * HPC + i] for i in range(HPC)])], axis=0))
        bv_c = np.ascontiguousarray(b_v[hs])
        wo_c = np.ascontiguousarray(
            (W_o[hs] * (g[hs, None, None] / H)).astype(np.float16))
        selc_c = np.zeros((128, 128), np.float16)
        selc_c[0, 0:64] = 1.0
        selc_c[1, 64:128] = 1.0
        sel64_c = np.zeros((128, 2), np.float32)
        sel64_c[0:64, 0] = 1.0
        sel64_c[64:128, 1] = 1.0
        m = dict(xT=xT, wqkv=wqkv_c, bqk=bqk_c, bv=bv_c, wo=wo_c,
                 selc=selc_c, sel64d=sel64_c)
        if with_mask:
            m["maskT"] = np.ascontiguousarray(attn_mask.T)
        in_maps.append(m)
    return in_maps


def kernel(hidden_states, attn_mask, W_q, b_q, W_k, b_k, W_v, b_v, W_o, b_o, gate):
    hidden_states = np.asarray(hidden_states, dtype=np.float32)
    attn_mask = np.asarray(attn_mask, dtype=np.float32)
    W_q, b_q = np.asarray(W_q, np.float32), np.asarray(b_q, np.float32)
    W_k, b_k = np.asarray(W_k, np.float32), np.asarray(b_k, np.float32)
    W_v, b_v = np.asarray(W_v, np.float32), np.asarray(b_v, np.float32)
    W_o, b_o = np.asarray(W_o, np.float32), np.asarray(b_o, np.float32)
    gate = np.asarray(gate, np.float32)

    with_mask = bool(np.any(attn_mask))
    nc = _get_nc(with_mask)
    in_maps = _make_in_maps(hidden_states, attn_mask, W_q, b_q, W_k, b_k,
                            W_v, b_v, W_o, b_o, gate, with_mask)

    res = run_bass_kernel_spmd(nc, in_maps, core_ids=list(range(NCORES)))
    if res.exec_time_ns is not None:
        print(f"HW exec time: {res.exec_time_ns} ns")

    out = np.zeros((NT, D), dtype=np.float32)
    for r in res.results:
        out += r["outp"].astype(np.float32)
    b_eff = (np.clip(gate, 0.0, 1.0)[:, None] * b_o).sum(axis=0) / H
    out += b_eff[None, :]
    return out.reshape(B, T, D)
